# revision 26
# baseline (speedup 1.0000x reference)
"""CapsNet (nn_CapsNetBasic) forward pass as a Bass/Tile kernel on 8 TRN2 cores.

Sharding: 8 cores = 2 batch samples x 4 row-blocks of 32 output rows each.
Every core computes its 32x128-pixel slab end-to-end:
  conv1 (5x5, 1->256, bf16 im2col matmul with fused valid-mask/bias rows)
  primary caps conv (5x5, 256->256) in fp8e4m3 via DoubleRow matmuls: one
    instruction contracts both 128-channel halves per tap (25 matmuls/chain
    instead of 50) -- activations scaled x8, weights x64, rescaled in the
    preact activation (1/16384).
  per-capsule squash (partition-group reductions via 0/1 indicator matmuls)
  seg caps accumulated across 4 row-blocks into one PSUM tile at partition
    offsets {0,32,64,96} so the scalar tail (seg squash, length, masking,
    sigmoid) runs once per superblock on batched [4,512]/[128,512] tiles.
  recon 1x1 convs (16->64->128->1) per block, sigmoid via odd polynomial.
Superblock-0's tail matmuls are interleaved between superblock-1's primary
chains to keep the PE queue streaming; the final tail runs in two column
halves to shorten the drain. Routing softmaxes are constant for these shapes
(uniform 1/32 and singleton 1.0), so routing reduces to fixed reductions.
"""

import sys

sys.path.insert(0, "/opt/trn_rl_repo")

import numpy as np
import ml_dtypes
from contextlib import ExitStack

import concourse.bass as bass
import concourse.tile as tile
from concourse import mybir, bacc
from concourse.bass_utils import run_bass_kernel_spmd

F32 = mybir.dt.float32
F32R = mybir.dt.float32r
BF16 = mybir.dt.bfloat16
F8 = mybir.dt.float8e4
AF = mybir.ActivationFunctionType
DRMODE = mybir.MatmulPerfMode.DoubleRow
ADD = mybir.AluOpType.add
MULT = mybir.AluOpType.mult

B = 2
H = W = 128
RB = 32          # output rows per core
NBLK = 4         # row blocks per sample
NCORES = 8
RR = RB + 4      # conv1 buffer rows (halo 2 each side)
CW = W + 4       # padded width
AFLAT = RR * CW  # 4752
NPX = RB * W     # 4096 output pixels per core
QW = AFLAT // 4  # 1188 = 9 rows per conv1 quarter

SA = 8.0         # conv1-activation fp8 scale
SW = 64.0        # primary-conv weight fp8 scale
PSCALE = 1.0 / (32.0 * SA * SW)   # preact = psum*PSCALE + cb1

NP_BF16 = ml_dtypes.bfloat16
NP_F8 = ml_dtypes.float8_e4m3

INPUT_SHAPES = {
    "A4": (128, QW),              # bf16 im2col quarters
    "W1T4": (128, 256),           # bf16 conv1 weights (x SA)
    "WT8": (128, 2, 25, 256),     # fp8 primary conv weights (x SW)
    "YV": (NPX,),                 # f32 labels
    "PACKR": (128, 1024),         # f32r matmul-constant pack
    "PACKB": (128, 644),          # bf16 matmul-constant pack
    "PACKF": (128, 9),            # f32 bias/eps pack
}

_PROGRAM = None


def _build_program():
    nc = bacc.Bacc("TRN2", target_bir_lowering=False, debug=False, num_devices=NCORES)

    d = {}
    dts = {"A4": BF16, "W1T4": BF16, "WT8": F8, "YV": F32,
           "PACKR": F32R, "PACKB": BF16, "PACKF": F32}
    for name, shape in INPUT_SHAPES.items():
        d[name] = nc.dram_tensor(name, list(shape), dts[name], kind="ExternalInput").ap()
    for name in ("OSEG", "OREC"):
        d[name] = nc.dram_tensor(name, [NPX], BF16, kind="ExternalOutput").ap()

    with tile.TileContext(nc) as tc, ExitStack() as ctx:
        pers = ctx.enter_context(tc.tile_pool(name="pers", bufs=1))
        pa = ctx.enter_context(tc.tile_pool(name="act", bufs=3))
        pb = ctx.enter_context(tc.tile_pool(name="bft", bufs=4))
        pt = ctx.enter_context(tc.tile_pool(name="tsm", bufs=4))
        ppc = ctx.enter_context(tc.tile_pool(name="ppc", bufs=2, space="PSUM"))
        pps = ctx.enter_context(tc.tile_pool(name="pps", bufs=2, space="PSUM"))

        # ---- persistent loads. sync queue: conv1-critical + half the fp8
        # weights; scalar queue: const packs + the other half.
        A4 = pers.tile([128, QW], BF16, tag="A4")
        nc.sync.dma_start(A4[:], d["A4"][:])
        W1T4 = pers.tile([128, 256], BF16, tag="W1T4")
        nc.sync.dma_start(W1T4[:], d["W1T4"][:])
        PACKB = pers.tile([128, 644], BF16, tag="PACKB")
        nc.sync.dma_start(PACKB[:], d["PACKB"][:])
        PACKR = pers.tile([128, 1024], F32R, tag="PACKR")
        nc.scalar.dma_start(PACKR[:], d["PACKR"][:])
        PACKF = pers.tile([128, 9], F32, tag="PACKF")
        nc.scalar.dma_start(PACKF[:], d["PACKF"][:])

        WT8 = pers.tile([128, 2, 25, 256], F8, tag="WT8")
        for dy in range(5):
            eng = nc.sync if dy % 2 == 0 else nc.scalar
            eng.dma_start(WT8[:, :, 5 * dy:5 * dy + 5, :],
                          d["WT8"][:, :, 5 * dy:5 * dy + 5, :])

        # const views (all matmul outputs land at partition 0 or use
        # 32-aligned row bands; cross-partition placement happens via
        # zero-padded lhsT columns + psum accumulation)
        IND2a = PACKR[0:16, 0:128]       # capsule->atom broadcast, m=0 rows
        IND2b = PACKR[32:48, 0:128]      # copy for m=1 rows
        BCIND = PACKR[0:4, 128:256]      # block->group broadcast
        WR1T = [PACKR[32 * j:32 * j + 16, 256:384] for j in range(4)]
        WR2T = PACKR[:, 384:512]         # rows 64-127 zero
        WR3T = [PACKR[:, 512 + 128 * j:640 + 128 * j] for j in range(4)]
        INDSQ = [PACKB[:, 64 * m:64 * m + 64] for m in range(2)]
        WsT4 = [PACKB[:, 128 + 128 * j:256 + 128 * j] for j in range(4)]
        INDSEG = PACKB[0:112, 640:644]   # group->block 0/1 reduction
        CB1 = PACKF[:, 0:2]
        CB2V = PACKF[0:112, 2:3]
        EPS48 = PACKF[0:48, 3:4]
        EPS4 = PACKF[0:4, 4:5]
        BR1 = PACKF[:, 5:6]              # rows 64-127 zero
        BR2 = PACKF[:, 6:7]
        BR3V = PACKF[0:97, 7:8]          # br3 at rows {0,32,64,96}

        # ---- conv1: 1->256 5x5 via host im2col (25 taps + valid-mask + bias
        # rows), 4 column-quarters on PE row bands {0,32,64,96}. relu out in
        # fp8 (x SA folded into W1T4), split across ACT (m=0) and Pool (m=1).
        C1 = pers.tile([128, 2, RR, CW], F8, tag="C1")
        C1F = [C1[:, m, :, :].rearrange("p r c -> p (r c)") for m in range(2)]
        for qt in range(4):
            for m in range(2):
                for qoff in range(0, QW, 512):
                    n = min(512, QW - qoff)
                    ps = ppc.tile([128, 512], F32, tag="ppc")
                    nc.tensor.matmul(
                        ps[:, :n],
                        W1T4[32 * qt:32 * qt + 27, m * 128:(m + 1) * 128],
                        A4[32 * qt:32 * qt + 27, qoff:qoff + n],
                        start=True, stop=True,
                        tile_position=(32 * qt, 0),
                    )
                    dst = C1F[m][:, QW * qt + qoff:QW * qt + qoff + n]
                    if (qt + m) % 2 == 0:
                        nc.scalar.activation(dst, ps[:, :n], AF.Relu,
                                             bias=0.0, scale=1.0)
                    else:
                        nc.vector.tensor_scalar_max(dst, ps[:, :n], 0.0)

        N = 512

        psp = ctx.enter_context(tc.tile_pool(name="psp", bufs=2, space="PSUM"))

        bst = {}

        def block_front(row0):
            """Primary DR conv + capsule square/reduce + squash scalar chain
            for one 4-row block. Both m halves' |s|^2 land in one SQ64 psum
            tile (m0 at partitions 0-15, m1 at 32-47) via zero-padded
            accumulating indicator matmuls."""
            st = {}
            bst[row0] = st
            P = [None, None]
            for m in range(2):
                ps = ppc.tile([128, 512], F32, tag="ppc")
                for t in range(25):
                    dy, dx = divmod(t, 5)
                    nc.tensor.matmul(
                        ps[:, :N],
                        WT8[:, :, t, m * 128:(m + 1) * 128],
                        C1[:, :, row0 + dy:row0 + dy + 4, dx:dx + 128],
                        start=(t == 0), stop=(t == 24),
                        perf_mode=DRMODE,
                    )
                P[m] = pb.tile([128, 512], BF16, tag="P", name="P")
                nc.scalar.activation(P[m][:, :N], ps[:, :N], AF.Identity,
                                     bias=CB1[:, m:m + 1], scale=PSCALE)
            st["P"] = P
            sq = pps.tile([64, 512], F32, tag="pps", name="sq")
            for m in range(2):
                S = pb.tile([128, 512], BF16, tag="S", name="S")
                nc.vector.tensor_mul(out=S[:, :N], in0=P[m][:, :N],
                                     in1=P[m][:, :N])
                nc.tensor.matmul(sq[0:64, :N], INDSQ[m][:], S[:, :N],
                                 start=(m == 0), stop=(m == 1))
            tq = pt.tile([48, 512], F32, tag="tq")
            nc.scalar.activation(tq[:, :N], sq[0:48, :N], AF.Sqrt,
                                 bias=EPS48[:], scale=1.0)
            u = pt.tile([48, 512], F32, tag="u")
            nc.vector.scalar_tensor_tensor(
                out=u[:, :N], in0=sq[0:48, :N], scalar=1.0, in1=tq[:, :N],
                op0=ADD, op1=MULT)
            rf0 = pt.tile([48, 512], F32, tag="rf0")
            nc.vector.reciprocal_approx_fast(out=rf0[:, :N], in_=u[:, :N])
            rf = pt.tile([48, 512], F32R, tag="rf")
            st["rf"] = rf
            nc.vector.tensor_mul(out=rf[:, :N], in0=sq[0:48, :N],
                                 in1=rf0[:, :N])

        def block_back(row0, sbst):
            """Broadcast squash factors, apply, seg conv accumulating into
            the superblock's SPP128 (block j at partitions 32j via
            zero-padded WsT columns). Emitted one block behind the fronts
            so the PE never waits on the squash chain."""
            st = bst.pop(row0)
            j = (row0 % 16) // 4
            if j == 0:
                sbst["spp"] = psp.tile([128, 512], F32, tag="spp", name="spp")
            spp = sbst["spp"]
            for m in range(2):
                bc = pps.tile([128, 512], F32, tag="pps", name="bc")
                nc.tensor.matmul(bc[:, :N], IND2a[:] if m == 0 else IND2b[:],
                                 st["rf"][32 * m:32 * m + 16, :N],
                                 start=True, stop=True,
                                 tile_position=(32 * m, 0))
                pm = pb.tile([128, 512], BF16, tag="pm")
                nc.vector.tensor_mul(out=pm[:, :N], in0=st["P"][m][:, :N],
                                     in1=bc[:, :N])
                nc.tensor.matmul(spp[:, :N], WsT4[j][:], pm[:, :N],
                                 start=(j == 0 and m == 0),
                                 stop=(j == 3 and m == 1))

        # ---- superblock tail (pixel range [p0, p0+2048), blocks at
        # partition groups 32j of SPP128/R3P4). Stages interleave with later
        # blocks' fronts/backs.
        def tail_a(st, n0, n1):
            """seg preact + squash scalars; cols [n0,n1)."""
            st["sp4"] = pb.tile([128, 512], BF16, tag="sp4", name="sp4")
            nc.scalar.activation(st["sp4"][0:112, n0:n1],
                                 st["spp"][0:112, n0:n1],
                                 AF.Identity, bias=CB2V[:], scale=1.0)
            sp2 = pb.tile([128, 512], BF16, tag="sp2")
            nc.vector.tensor_mul(out=sp2[0:112, n0:n1],
                                 in0=st["sp4"][0:112, n0:n1],
                                 in1=st["sp4"][0:112, n0:n1])
            sq2 = pps.tile([4, 512], F32, tag="pps", name="sq2")
            st["sq2"] = sq2
            nc.tensor.matmul(sq2[0:4, n0:n1], INDSEG[:], sp2[0:112, n0:n1],
                             start=True, stop=True)
            t2 = pt.tile([4, 512], F32, tag="t2")
            st["t2"] = t2
            nc.scalar.activation(t2[:, n0:n1], sq2[0:4, n0:n1], AF.Sqrt,
                                 bias=EPS4[:], scale=1.0)
            ytq = pt.tile([4, 512], F32, tag="ytq")
            st["ytq"] = ytq
            nc.vector.tensor_mul(out=ytq[:, n0:n1], in0=sq2[0:4, n0:n1],
                                 in1=st["yt"][:, n0:n1])
            u2 = pt.tile([4, 512], F32, tag="u2")
            nc.vector.scalar_tensor_tensor(
                out=u2[:, n0:n1], in0=sq2[0:4, n0:n1], scalar=1.0,
                in1=t2[:, n0:n1], op0=ADD, op1=MULT)
            f2a = pt.tile([4, 512], F32, tag="f2a")
            st["f2a"] = f2a
            nc.vector.reciprocal_approx_fast(out=f2a[:, n0:n1], in_=u2[:, n0:n1])
            f2 = pt.tile([4, 512], F32, tag="f2")
            st["f2"] = f2
            nc.vector.tensor_mul(out=f2[:, n0:n1], in0=sq2[0:4, n0:n1],
                                 in1=f2a[:, n0:n1])

        def tail_b(st, n0, n1):
            """length out, mask broadcast; cols [n0,n1)."""
            p0 = st["p0"]
            oseg = pt.tile([4, 512], BF16, tag="oseg")
            nc.vector.tensor_mul(out=oseg[:, n0:n1], in0=st["f2"][:, n0:n1],
                                 in1=st["t2"][:, n0:n1])
            nc.sync.dma_start(
                d["OSEG"][p0:p0 + 2048].rearrange("(p n) -> p n", p=4)[:, n0:n1],
                oseg[:, n0:n1])
            m1v = pt.tile([4, 512], F32R, tag="m1v")
            nc.vector.tensor_mul(out=m1v[:, n0:n1], in0=st["f2a"][:, n0:n1],
                                 in1=st["ytq"][:, n0:n1])
            bmp = pps.tile([128, 512], F32, tag="pps", name="bmp")
            nc.tensor.matmul(bmp[:, n0:n1], BCIND[:], m1v[:, n0:n1],
                             start=True, stop=True)
            nc.vector.tensor_mul(out=st["masked"][0:112, n0:n1],
                                 in0=st["sp4"][0:112, n0:n1],
                                 in1=bmp[0:112, n0:n1])

        def tail_r1(st, j, n0, n1):
            """recon stage 1 for block j; cols [n0,n1)."""
            r1p = pps.tile([128, 512], F32, tag="pps", name="r1p")
            nc.tensor.matmul(r1p[:, n0:n1], WR1T[j][:],
                             st["masked"][32 * j:32 * j + 16, n0:n1],
                             start=True, stop=True,
                             tile_position=(32 * j, 0))
            r1 = pa.tile([128, 512], F32R, tag="r1", name="r1", bufs=8)
            st[f"r1_{j}"] = r1
            nc.scalar.activation(r1[:, n0:n1], r1p[:, n0:n1], AF.Relu,
                                 bias=BR1[:], scale=1.0)

        def tail_r2(st, j, n0, n1):
            """recon stages 2+3 for block j; r3 accumulates into R3P4 at
            partition 32j via zero-padded WR3T columns."""
            r1 = st[f"r1_{j // 2}"]
            r2p = pps.tile([128, 512], F32, tag="pps", name="r2p")
            nc.tensor.matmul(r2p[:, n0:n1], W2V[j % 2][:], r1[:, n0:n1],
                             start=True, stop=True)
            r2 = pa.tile([128, 512], F32R, tag="r2")
            nc.vector.tensor_scalar(out=r2[:, n0:n1], in0=r2p[:, n0:n1],
                                    scalar1=BR2[:], scalar2=0.0,
                                    op0=ADD, op1=mybir.AluOpType.max)
            nc.tensor.matmul(st["r3p"][:, n0:n1], WR3T[j][:], r2[:, n0:n1],
                             start=(j == 0), stop=(j == 3))

        def tail_sig(st, n0, n1):
            """sigmoid(x) ~= 0.5 + x(1/4 + x2(-1/48 + x2/480)) on R3P4
            [97,512] (rows {0,32,64,96}); 4 output DMAs."""
            p0 = st["p0"]
            r3p = st["r3p"]
            xv = pb.tile([97, 512], BF16, tag="xv")
            nc.vector.tensor_scalar(out=xv[:, n0:n1], in0=r3p[0:97, n0:n1],
                                    scalar1=BR3V[:], scalar2=None, op0=ADD)
            x2 = pb.tile([97, 512], BF16, tag="x2")
            nc.vector.tensor_mul(out=x2[:, n0:n1], in0=xv[:, n0:n1],
                                 in1=xv[:, n0:n1])
            x3 = pb.tile([97, 512], BF16, tag="x3")
            nc.vector.tensor_mul(out=x3[:, n0:n1], in0=xv[:, n0:n1],
                                 in1=x2[:, n0:n1])
            v = pb.tile([97, 512], BF16, tag="v")
            nc.vector.tensor_scalar(out=v[:, n0:n1], in0=x2[:, n0:n1],
                                    scalar1=1.0 / 480.0, scalar2=-1.0 / 48.0,
                                    op0=MULT, op1=ADD)
            r = pb.tile([97, 512], BF16, tag="r")
            nc.vector.tensor_scalar(out=r[:, n0:n1], in0=xv[:, n0:n1],
                                    scalar1=0.25, scalar2=0.5,
                                    op0=MULT, op1=ADD)
            w = pb.tile([97, 512], BF16, tag="w")
            nc.vector.tensor_mul(out=w[:, n0:n1], in0=x3[:, n0:n1],
                                 in1=v[:, n0:n1])
            orec = pb.tile([97, 512], BF16, tag="orec")
            nc.vector.tensor_tensor(out=orec[:, n0:n1], in0=w[:, n0:n1],
                                    in1=r[:, n0:n1], op=ADD)
            for j, eng in enumerate((nc.sync, nc.gpsimd, nc.scalar, nc.gpsimd)):
                eng.dma_start(
                    d["OREC"][p0 + 512 * j:p0 + 512 * j + 512]
                    .rearrange("(p n) -> p n", p=1)[:, n0:n1],
                    orec[32 * j:32 * j + 1, n0:n1])

        def sb_state(sb):
            st = {"p0": 2048 * sb, "sb": sb}
            st["masked"] = pa.tile([128, 512], F32R, tag="masked", name="masked")
            st["r3p"] = psp.tile([128, 512], F32, tag="r3p", name="r3p")
            st["yt"] = pt.tile([4, 512], F32, tag="yt", name="yt")
            nc.sync.dma_start(
                st["yt"][:],
                d["YV"][st["p0"]:st["p0"] + 2048].rearrange("(p n) -> p n", p=4))
            return st

        # ---- schedule: block fronts run one block ahead of their backs so
        # the PE never stalls on the squash chain; sb0's tail interleaves
        # sb1's blocks; sb1's tail runs in two column halves.
        st0 = sb_state(0)
        st1 = sb_state(1)
        block_front(0)
        block_front(4)
        block_back(0, st0)
        block_front(8)
        block_back(4, st0)
        block_front(12)
        block_back(8, st0)
        block_front(16)
        block_back(12, st0)
        tail_a(st0, 0, 512)
        block_front(20)
        block_back(16, st1)
        tail_b(st0, 0, 512)
        block_front(24)
        block_back(20, st1)
        for j in range(4):
            tail_r1(st0, j, 0, 512)
        block_front(28)
        block_back(24, st1)
        for j in range(4):
            tail_r2(st0, j, 0, 512)
        block_back(28, st1)
        tail_sig(st0, 0, 512)
        ha, hb = dict(st1), dict(st1)
        tail_a(ha, 0, 256)
        tail_a(hb, 256, 512)
        tail_b(ha, 0, 256)
        tail_b(hb, 256, 512)
        for j in range(4):
            tail_r1(st1, j, 0, 512)
        for j in range(4):
            tail_r2(st1, j, 0, 512)
        tail_sig(st1, 0, 512)

    nc.compile()
    return nc


def _get_program():
    global _PROGRAM
    if _PROGRAM is None:
        _PROGRAM = _build_program()
    return _PROGRAM


def _host_prep(inputs):
    """Build per-core input maps from the full problem inputs."""
    x = np.asarray(inputs["x"], np.float32)
    y = np.asarray(inputs["y"], np.float32)
    W1 = np.asarray(inputs["W1"], np.float32)
    b1 = np.asarray(inputs["b1"], np.float32)
    Wp = np.asarray(inputs["Wp"], np.float32)
    bp = np.asarray(inputs["bp"], np.float32)
    cbp = np.asarray(inputs["cbp"], np.float32)
    Ws = np.asarray(inputs["Ws"], np.float32)
    bs = np.asarray(inputs["bs"], np.float32)
    cbs = np.asarray(inputs["cbs"], np.float32)
    Wr1 = np.asarray(inputs["Wr1"], np.float32)
    br1 = np.asarray(inputs["br1"], np.float32)
    Wr2 = np.asarray(inputs["Wr2"], np.float32)
    br2 = np.asarray(inputs["br2"], np.float32)
    Wr3 = np.asarray(inputs["Wr3"], np.float32)
    br3 = np.asarray(inputs["br3"], np.float32)

    W1r = W1.reshape(256, 25).T                      # [25 tap, 256 oc]
    W1T = np.concatenate([W1r, np.ones((1, 256), np.float32),
                          b1[None, :]], axis=0) * SA  # [27, 256], x SA
    W1T4 = np.zeros((128, 256), np.float32)
    for qt in range(4):
        W1T4[32 * qt:32 * qt + 27] = W1T
    WT8 = (np.ascontiguousarray(
        Wp.reshape(256, 2, 128, 25).transpose(1, 3, 2, 0)) * SW
    ).reshape(2, 25, 128, 256).transpose(2, 0, 1, 3)  # [128p, 2k, 25t, 256oc]
    WT8 = np.ascontiguousarray(WT8).astype(NP_F8)

    oc = np.arange(128)
    WsT = np.ascontiguousarray(Ws.reshape(16, 8).T[oc % 8])       # [128, 16]
    IND2 = (np.arange(128)[None, :] // 8 == np.arange(16)[:, None]).astype(np.float32)
    INDSQ = np.ascontiguousarray(IND2.T)
    cb1 = np.empty((128, 2), np.float32)
    for m in range(2):
        g = m * 128 + np.arange(128)
        cb1[:, m] = bp[g] / 32.0 + cbp[g // 8, g % 8, 0, 0]
    cb2 = (32.0 * bs + cbs[0, :, 0, 0]).astype(np.float32)

    packr = np.zeros((128, 1024), np.float32)
    packr[0:16, 0:128] = IND2
    packr[32:48, 0:128] = IND2
    for j in range(4):
        packr[j, 128 + 32 * j:128 + 32 * j + 16] = 1.0   # BCIND
        packr[32 * j:32 * j + 16, 256:320] = Wr1.reshape(64, 16).T
        packr[:, 512 + 128 * j + 32 * j] = Wr3.reshape(128)  # WR3T_j
    packr[0:64, 384:512] = Wr2.reshape(128, 64).T        # WR2T (rows 64+ zero)

    packb = np.zeros((128, 644), np.float32)
    packb[:, 0:16] = INDSQ                               # INDSQ m=0 -> parts 0-15
    packb[:, 96:112] = INDSQ                             # INDSQ m=1 -> parts 32-47
    for j in range(4):
        packb[:, 128 + 128 * j + 32 * j:128 + 128 * j + 32 * j + 16] = WsT
        packb[32 * j:32 * j + 16, 640 + j] = 1.0         # INDSEG

    packf = np.zeros((128, 9), np.float32)
    packf[:, 0:2] = cb1
    for j in range(4):
        packf[32 * j:32 * j + 16, 2] = cb2               # CB2V
    packf[0:48, 3] = 1e-9                                # EPS48
    packf[0:4, 4] = 1e-9                                 # EPS4
    packf[0:64, 5] = br1
    packf[:, 6] = br2
    packf[0:97:32, 7] = br3[0]                           # BR3V

    shared = {
        "W1T4": W1T4.astype(NP_BF16),
        "WT8": WT8,
        "PACKR": packr,
        "PACKB": packb.astype(NP_BF16),
        "PACKF": packf,
    }

    in_maps = []
    for c in range(NCORES):
        b, j = divmod(c, NBLK)
        r0 = RB * j
        xpad = np.zeros((H + 8, W + 8), np.float32)
        xpad[4:4 + H, 4:4 + W] = x[b, 0]
        A = np.empty((27, RR, CW), np.float32)
        for dy in range(5):
            for dx in range(5):
                A[dy * 5 + dx] = xpad[r0 + dy:r0 + dy + RR, dx:dx + CW]
        # valid-mask row: -1e30 where the conv1 output position is padding
        rr = np.arange(RR)[:, None]
        cc = np.arange(CW)[None, :]
        valid = (r0 - 2 + rr >= 0) & (r0 - 2 + rr < H) & (cc >= 2) & (cc < 2 + W)
        A[25] = np.where(valid, 0.0, -1e30).astype(np.float32)
        A[26] = 1.0
        m = dict(shared)
        Af = A.reshape(27, AFLAT)
        A4 = np.zeros((128, QW), np.float32)
        for qt in range(4):
            A4[32 * qt:32 * qt + 27] = Af[:, QW * qt:QW * (qt + 1)]
        m["A4"] = A4.astype(NP_BF16)
        m["YV"] = np.ascontiguousarray(y[b, 0, r0:r0 + RB, :].reshape(NPX))
        in_maps.append(m)
    return in_maps


def _gather(results):
    out_seg = np.empty((B, 1, H, W), np.float32)
    out_rec = np.empty((B, 1, H, W), np.float32)
    for c in range(NCORES):
        b, j = divmod(c, NBLK)
        r0 = RB * j
        out_seg[b, 0, r0:r0 + RB, :] = \
            results[c]["OSEG"].astype(np.float32).reshape(RB, W)
        out_rec[b, 0, r0:r0 + RB, :] = \
            results[c]["OREC"].astype(np.float32).reshape(RB, W)
    return out_seg, out_rec


def kernel(**inputs):
    nc = _get_program()
    in_maps = _host_prep(inputs)
    res = run_bass_kernel_spmd(nc, in_maps, list(range(NCORES)))
    return _gather(res.results)


# revision 27
# speedup vs baseline: 1.0320x; 1.0320x over previous
"""CapsNet (nn_CapsNetBasic) forward pass as a Bass/Tile kernel on 8 TRN2 cores.

Sharding: 8 cores = 2 batch samples x 4 row-blocks of 32 output rows each.
Every core computes its 32x128-pixel slab end-to-end:
  conv1 (5x5, 1->256, bf16 im2col matmul with fused valid-mask/bias rows)
  primary caps conv (5x5, 256->256) in fp8e4m3 via DoubleRow matmuls: one
    instruction contracts both 128-channel halves per tap (25 matmuls/chain
    instead of 50) -- activations scaled x8, weights x64, rescaled in the
    preact activation (1/16384).
  per-capsule squash (partition-group reductions via 0/1 indicator matmuls)
  seg caps accumulated across 4 row-blocks into one PSUM tile at partition
    offsets {0,32,64,96} so the scalar tail (seg squash, length, masking,
    sigmoid) runs once per superblock on batched [4,512]/[128,512] tiles.
  recon 1x1 convs (16->64->128->1) per block, sigmoid via odd polynomial.
Superblock-0's tail matmuls are interleaved between superblock-1's primary
chains to keep the PE queue streaming; the final tail runs in two column
halves to shorten the drain. Routing softmaxes are constant for these shapes
(uniform 1/32 and singleton 1.0), so routing reduces to fixed reductions.
"""

import sys

sys.path.insert(0, "/opt/trn_rl_repo")

import numpy as np
import ml_dtypes
from contextlib import ExitStack

import concourse.bass as bass
import concourse.tile as tile
from concourse import mybir, bacc
from concourse.bass_utils import run_bass_kernel_spmd

F32 = mybir.dt.float32
F32R = mybir.dt.float32r
BF16 = mybir.dt.bfloat16
F8 = mybir.dt.float8e4
AF = mybir.ActivationFunctionType
DRMODE = mybir.MatmulPerfMode.DoubleRow
ADD = mybir.AluOpType.add
MULT = mybir.AluOpType.mult

B = 2
H = W = 128
RB = 32          # output rows per core
NBLK = 4         # row blocks per sample
NCORES = 8
RR = RB + 4      # conv1 buffer rows (halo 2 each side)
CW = W + 4       # padded width
AFLAT = RR * CW  # 4752
NPX = RB * W     # 4096 output pixels per core
QW = AFLAT // 4  # 1188 = 9 rows per conv1 quarter

SA = 8.0         # conv1-activation fp8 scale
SW = 64.0        # primary-conv weight fp8 scale
PSCALE = 1.0 / (32.0 * SA * SW)   # preact = psum*PSCALE + cb1

NP_BF16 = ml_dtypes.bfloat16
NP_F8 = ml_dtypes.float8_e4m3

INPUT_SHAPES = {
    "A4": (128, QW),              # bf16 im2col quarters
    "W1T4": (128, 256),           # bf16 conv1 weights (x SA)
    "WT8": (128, 2, 25, 256),     # fp8 primary conv weights (x SW)
    "YV": (NPX,),                 # f32 labels
    "PACKR": (128, 1024),         # f32r matmul-constant pack
    "PACKB": (128, 644),          # bf16 matmul-constant pack
    "PACKF": (128, 9),            # f32 bias/eps pack
}

_PROGRAM = None


def _build_program():
    nc = bacc.Bacc("TRN2", target_bir_lowering=False, debug=False, num_devices=NCORES)

    d = {}
    dts = {"A4": BF16, "W1T4": BF16, "WT8": F8, "YV": F32,
           "PACKR": F32R, "PACKB": BF16, "PACKF": F32}
    for name, shape in INPUT_SHAPES.items():
        d[name] = nc.dram_tensor(name, list(shape), dts[name], kind="ExternalInput").ap()
    for name in ("OSEG", "OREC"):
        d[name] = nc.dram_tensor(name, [NPX], BF16, kind="ExternalOutput").ap()

    with tile.TileContext(nc) as tc, ExitStack() as ctx:
        pers = ctx.enter_context(tc.tile_pool(name="pers", bufs=1))
        pa = ctx.enter_context(tc.tile_pool(name="act", bufs=3))
        pb = ctx.enter_context(tc.tile_pool(name="bft", bufs=4))
        pt = ctx.enter_context(tc.tile_pool(name="tsm", bufs=4))
        ppc = ctx.enter_context(tc.tile_pool(name="ppc", bufs=2, space="PSUM"))
        pps = ctx.enter_context(tc.tile_pool(name="pps", bufs=2, space="PSUM"))

        # ---- persistent loads. sync queue: conv1-critical + half the fp8
        # weights; scalar queue: const packs + the other half.
        A4 = pers.tile([128, QW], BF16, tag="A4")
        nc.sync.dma_start(A4[:], d["A4"][:])
        W1T4 = pers.tile([128, 256], BF16, tag="W1T4")
        nc.sync.dma_start(W1T4[:], d["W1T4"][:])
        PACKB = pers.tile([128, 644], BF16, tag="PACKB")
        nc.sync.dma_start(PACKB[:], d["PACKB"][:])
        PACKR = pers.tile([128, 1024], F32R, tag="PACKR")
        nc.scalar.dma_start(PACKR[:], d["PACKR"][:])
        PACKF = pers.tile([128, 9], F32, tag="PACKF")
        nc.scalar.dma_start(PACKF[:], d["PACKF"][:])

        WT8 = pers.tile([128, 2, 25, 256], F8, tag="WT8")
        for dy in range(5):
            eng = nc.sync if dy % 2 == 0 else nc.scalar
            eng.dma_start(WT8[:, :, 5 * dy:5 * dy + 5, :],
                          d["WT8"][:, :, 5 * dy:5 * dy + 5, :])

        # const views (all matmul outputs land at partition 0 or use
        # 32-aligned row bands; cross-partition placement happens via
        # zero-padded lhsT columns + psum accumulation)
        IND2a = PACKR[0:16, 0:128]       # capsule->atom broadcast, m=0 rows
        IND2b = PACKR[32:48, 0:128]      # copy for m=1 rows
        BCIND = PACKR[0:4, 128:256]      # block->group broadcast
        WR1T = [PACKR[32 * j:32 * j + 16, 256:384] for j in range(4)]
        WR2T = PACKR[:, 384:512]         # rows 64-127 zero
        WR3T = [PACKR[:, 512 + 128 * j:640 + 128 * j] for j in range(4)]
        INDSQ = [PACKB[:, 64 * m:64 * m + 64] for m in range(2)]
        WsT4 = [PACKB[:, 128 + 128 * j:256 + 128 * j] for j in range(4)]
        INDSEG = PACKB[0:112, 640:644]   # group->block 0/1 reduction
        CB1 = PACKF[:, 0:2]
        CB2V = PACKF[0:112, 2:3]
        EPS48 = PACKF[0:48, 3:4]
        EPS4 = PACKF[0:4, 4:5]
        BR1 = PACKF[:, 5:6]              # rows 64-127 zero
        BR2 = PACKF[:, 6:7]
        BR3V = PACKF[0:97, 7:8]          # br3 at rows {0,32,64,96}

        # ---- conv1: 1->256 5x5 via host im2col (25 taps + valid-mask + bias
        # rows), 4 column-quarters on PE row bands {0,32,64,96}. relu out in
        # fp8 (x SA folded into W1T4), split across ACT (m=0) and Pool (m=1).
        C1 = pers.tile([128, 2, RR, CW], F8, tag="C1")
        C1F = [C1[:, m, :, :].rearrange("p r c -> p (r c)") for m in range(2)]
        for qt in range(4):
            for m in range(2):
                for qoff in range(0, QW, 512):
                    n = min(512, QW - qoff)
                    ps = ppc.tile([128, 512], F32, tag="ppc")
                    nc.tensor.matmul(
                        ps[:, :n],
                        W1T4[32 * qt:32 * qt + 27, m * 128:(m + 1) * 128],
                        A4[32 * qt:32 * qt + 27, qoff:qoff + n],
                        start=True, stop=True,
                        tile_position=(32 * qt, 0),
                    )
                    dst = C1F[m][:, QW * qt + qoff:QW * qt + qoff + n]
                    if (qt + m) % 2 == 0:
                        nc.scalar.activation(dst, ps[:, :n], AF.Relu,
                                             bias=0.0, scale=1.0)
                    else:
                        nc.vector.tensor_scalar_max(dst, ps[:, :n], 0.0)

        N = 512

        psp = ctx.enter_context(tc.tile_pool(name="psp", bufs=2, space="PSUM"))

        bst = {}

        def block_front(row0):
            """Primary DR conv + capsule square/reduce + squash scalar chain
            for one 4-row block. Both m halves' |s|^2 land in one SQ64 psum
            tile (m0 at partitions 0-15, m1 at 32-47) via zero-padded
            accumulating indicator matmuls."""
            st = {}
            bst[row0] = st
            P = [None, None]
            for m in range(2):
                ps = ppc.tile([128, 512], F32, tag="ppc")
                for t in range(25):
                    dy, dx = divmod(t, 5)
                    nc.tensor.matmul(
                        ps[:, :N],
                        WT8[:, :, t, m * 128:(m + 1) * 128],
                        C1[:, :, row0 + dy:row0 + dy + 4, dx:dx + 128],
                        start=(t == 0), stop=(t == 24),
                        perf_mode=DRMODE,
                    )
                P[m] = pb.tile([128, 512], BF16, tag="P", name="P")
                nc.scalar.activation(P[m][:, :N], ps[:, :N], AF.Identity,
                                     bias=CB1[:, m:m + 1], scale=PSCALE)
            st["P"] = P
            sq = pps.tile([64, 512], F32, tag="pps", name="sq")
            for m in range(2):
                S = pb.tile([128, 512], BF16, tag="S", name="S")
                nc.vector.tensor_mul(out=S[:, :N], in0=P[m][:, :N],
                                     in1=P[m][:, :N])
                nc.tensor.matmul(sq[0:64, :N], INDSQ[m][:], S[:, :N],
                                 start=(m == 0), stop=(m == 1))
            tq = pt.tile([48, 512], F32, tag="tq")
            nc.scalar.activation(tq[:, :N], sq[0:48, :N], AF.Sqrt,
                                 bias=EPS48[:], scale=1.0)
            u = pt.tile([48, 512], F32, tag="u")
            nc.vector.scalar_tensor_tensor(
                out=u[:, :N], in0=sq[0:48, :N], scalar=1.0, in1=tq[:, :N],
                op0=ADD, op1=MULT)
            rf0 = pt.tile([48, 512], F32, tag="rf0")
            nc.vector.reciprocal_approx_fast(out=rf0[:, :N], in_=u[:, :N])
            rf = pt.tile([48, 512], F32R, tag="rf")
            st["rf"] = rf
            nc.vector.tensor_mul(out=rf[:, :N], in0=sq[0:48, :N],
                                 in1=rf0[:, :N])

        def block_back(row0, sbst):
            """Broadcast squash factors, apply, seg conv accumulating into
            the superblock's SPP128 (block j at partitions 32j via
            zero-padded WsT columns). Emitted one block behind the fronts
            so the PE never waits on the squash chain."""
            st = bst.pop(row0)
            j = (row0 % 16) // 4
            if j == 0:
                sbst["spp"] = psp.tile([128, 512], F32, tag="spp", name="spp")
            spp = sbst["spp"]
            for m in range(2):
                bc = pps.tile([128, 512], F32, tag="pps", name="bc")
                nc.tensor.matmul(bc[:, :N], IND2a[:] if m == 0 else IND2b[:],
                                 st["rf"][32 * m:32 * m + 16, :N],
                                 start=True, stop=True,
                                 tile_position=(32 * m, 0))
                pm = pb.tile([128, 512], BF16, tag="pm")
                nc.vector.tensor_mul(out=pm[:, :N], in0=st["P"][m][:, :N],
                                     in1=bc[:, :N])
                nc.tensor.matmul(spp[:, :N], WsT4[j][:], pm[:, :N],
                                 start=(j == 0 and m == 0),
                                 stop=(j == 3 and m == 1))

        # ---- superblock tail (pixel range [p0, p0+2048), blocks at
        # partition groups 32j of SPP128/R3P4). Stages interleave with later
        # blocks' fronts/backs.
        def tail_a(st, n0, n1):
            """seg preact + squash scalars; cols [n0,n1)."""
            st["sp4"] = pb.tile([128, 512], BF16, tag="sp4", name="sp4")
            nc.scalar.activation(st["sp4"][0:112, n0:n1],
                                 st["spp"][0:112, n0:n1],
                                 AF.Identity, bias=CB2V[:], scale=1.0)
            sp2 = pb.tile([128, 512], BF16, tag="sp2")
            nc.vector.tensor_mul(out=sp2[0:112, n0:n1],
                                 in0=st["sp4"][0:112, n0:n1],
                                 in1=st["sp4"][0:112, n0:n1])
            sq2 = pps.tile([4, 512], F32, tag="pps", name="sq2")
            st["sq2"] = sq2
            nc.tensor.matmul(sq2[0:4, n0:n1], INDSEG[:], sp2[0:112, n0:n1],
                             start=True, stop=True)
            t2 = pt.tile([4, 512], F32, tag="t2")
            st["t2"] = t2
            nc.scalar.activation(t2[:, n0:n1], sq2[0:4, n0:n1], AF.Sqrt,
                                 bias=EPS4[:], scale=1.0)
            ytq = pt.tile([4, 512], F32, tag="ytq")
            st["ytq"] = ytq
            nc.vector.tensor_mul(out=ytq[:, n0:n1], in0=sq2[0:4, n0:n1],
                                 in1=st["yt"][:, n0:n1])
            u2 = pt.tile([4, 512], F32, tag="u2")
            nc.vector.scalar_tensor_tensor(
                out=u2[:, n0:n1], in0=sq2[0:4, n0:n1], scalar=1.0,
                in1=t2[:, n0:n1], op0=ADD, op1=MULT)
            f2a = pt.tile([4, 512], F32, tag="f2a")
            st["f2a"] = f2a
            nc.vector.reciprocal_approx_fast(out=f2a[:, n0:n1], in_=u2[:, n0:n1])
            f2 = pt.tile([4, 512], F32, tag="f2")
            st["f2"] = f2
            nc.vector.tensor_mul(out=f2[:, n0:n1], in0=sq2[0:4, n0:n1],
                                 in1=f2a[:, n0:n1])

        def tail_b(st, n0, n1):
            """length out, mask broadcast; cols [n0,n1)."""
            p0 = st["p0"]
            oseg = pt.tile([4, 512], BF16, tag="oseg")
            nc.vector.tensor_mul(out=oseg[:, n0:n1], in0=st["f2"][:, n0:n1],
                                 in1=st["t2"][:, n0:n1])
            nc.sync.dma_start(
                d["OSEG"][p0:p0 + 2048].rearrange("(p n) -> p n", p=4)[:, n0:n1],
                oseg[:, n0:n1])
            m1v = pt.tile([4, 512], F32R, tag="m1v")
            nc.vector.tensor_mul(out=m1v[:, n0:n1], in0=st["f2a"][:, n0:n1],
                                 in1=st["ytq"][:, n0:n1])
            bmp = pps.tile([128, 512], F32, tag="pps", name="bmp")
            nc.tensor.matmul(bmp[:, n0:n1], BCIND[:], m1v[:, n0:n1],
                             start=True, stop=True)
            nc.vector.tensor_mul(out=st["masked"][0:112, n0:n1],
                                 in0=st["sp4"][0:112, n0:n1],
                                 in1=bmp[0:112, n0:n1])

        def tail_r1(st, j, n0, n1):
            """recon stage 1 for block j; cols [n0,n1)."""
            r1p = pps.tile([128, 512], F32, tag="pps", name="r1p")
            nc.tensor.matmul(r1p[:, n0:n1], WR1T[j][:],
                             st["masked"][32 * j:32 * j + 16, n0:n1],
                             start=True, stop=True,
                             tile_position=(32 * j, 0))
            r1 = pa.tile([128, 512], F32R, tag="r1", name="r1", bufs=8)
            st[f"r1_{j}"] = r1
            nc.scalar.activation(r1[:, n0:n1], r1p[:, n0:n1], AF.Relu,
                                 bias=BR1[:], scale=1.0)

        def tail_r2(st, j, n0, n1):
            """recon stages 2+3 for block j; r3 accumulates into R3P4 at
            partition 32j via zero-padded WR3T columns."""
            r1 = st[f"r1_{j // 2}"]
            r2p = pps.tile([128, 512], F32, tag="pps", name="r2p")
            nc.tensor.matmul(r2p[:, n0:n1], W2V[j % 2][:], r1[:, n0:n1],
                             start=True, stop=True)
            r2 = pa.tile([128, 512], F32R, tag="r2")
            nc.vector.tensor_scalar(out=r2[:, n0:n1], in0=r2p[:, n0:n1],
                                    scalar1=BR2[:], scalar2=0.0,
                                    op0=ADD, op1=mybir.AluOpType.max)
            nc.tensor.matmul(st["r3p"][:, n0:n1], WR3T[j][:], r2[:, n0:n1],
                             start=(j == 0), stop=(j == 3))

        def tail_sig(st, n0, n1, table=False):
            """sigmoid on R3P4 [97,512] (rows {0,32,64,96}); 4 output DMAs.
            table=True uses the ACT Sigmoid table in ONE op -- only legal
            after the last Sqrt use (the table swap evicts sqrt); the
            ~1.3us table load has no data deps and hides under the recon
            matmuls. Mid-kernel tails keep the DVE polynomial."""
            p0 = st["p0"]
            r3p = st["r3p"]
            if table:
                orect = pb.tile([97, 512], BF16, tag="orect", name="orect")
                nc.scalar.activation(orect[:, n0:n1], r3p[0:97, n0:n1],
                                     AF.Sigmoid, bias=BR3V[:], scale=1.0)
                for j, eng in enumerate((nc.sync, nc.gpsimd, nc.scalar,
                                         nc.sync)):
                    eng.dma_start(
                        d["OREC"][p0 + 512 * j:p0 + 512 * j + 512]
                        .rearrange("(p n) -> p n", p=1)[:, n0:n1],
                        orect[32 * j:32 * j + 1, n0:n1])
                return
            xv = pb.tile([97, 512], BF16, tag="xv")
            nc.vector.tensor_scalar(out=xv[:, n0:n1], in0=r3p[0:97, n0:n1],
                                    scalar1=BR3V[:], scalar2=None, op0=ADD)
            x2 = pb.tile([97, 512], BF16, tag="x2")
            nc.vector.tensor_mul(out=x2[:, n0:n1], in0=xv[:, n0:n1],
                                 in1=xv[:, n0:n1])
            x3 = pb.tile([97, 512], BF16, tag="x3")
            nc.vector.tensor_mul(out=x3[:, n0:n1], in0=xv[:, n0:n1],
                                 in1=x2[:, n0:n1])
            v = pb.tile([97, 512], BF16, tag="v")
            nc.vector.tensor_scalar(out=v[:, n0:n1], in0=x2[:, n0:n1],
                                    scalar1=1.0 / 480.0, scalar2=-1.0 / 48.0,
                                    op0=MULT, op1=ADD)
            r = pb.tile([97, 512], BF16, tag="r")
            nc.vector.tensor_scalar(out=r[:, n0:n1], in0=xv[:, n0:n1],
                                    scalar1=0.25, scalar2=0.5,
                                    op0=MULT, op1=ADD)
            w = pb.tile([97, 512], BF16, tag="w")
            nc.vector.tensor_mul(out=w[:, n0:n1], in0=x3[:, n0:n1],
                                 in1=v[:, n0:n1])
            orec = pb.tile([97, 512], BF16, tag="orec")
            nc.vector.tensor_tensor(out=orec[:, n0:n1], in0=w[:, n0:n1],
                                    in1=r[:, n0:n1], op=ADD)
            for j, eng in enumerate((nc.sync, nc.gpsimd, nc.scalar, nc.gpsimd)):
                eng.dma_start(
                    d["OREC"][p0 + 512 * j:p0 + 512 * j + 512]
                    .rearrange("(p n) -> p n", p=1)[:, n0:n1],
                    orec[32 * j:32 * j + 1, n0:n1])

        def sb_state(sb):
            st = {"p0": 2048 * sb, "sb": sb}
            st["masked"] = pa.tile([128, 512], F32R, tag="masked", name="masked")
            st["r3p"] = psp.tile([128, 512], F32, tag="r3p", name="r3p")
            st["yt"] = pt.tile([4, 512], F32, tag="yt", name="yt")
            nc.sync.dma_start(
                st["yt"][:],
                d["YV"][st["p0"]:st["p0"] + 2048].rearrange("(p n) -> p n", p=4))
            return st

        # ---- schedule: block fronts run one block ahead of their backs so
        # the PE never stalls on the squash chain; sb0's tail interleaves
        # sb1's blocks; sb1's tail runs in two column halves.
        st0 = sb_state(0)
        st1 = sb_state(1)
        block_front(0)
        block_front(4)
        block_back(0, st0)
        block_front(8)
        block_back(4, st0)
        block_front(12)
        block_back(8, st0)
        block_front(16)
        block_back(12, st0)
        tail_a(st0, 0, 512)
        block_front(20)
        block_back(16, st1)
        tail_b(st0, 0, 512)
        block_front(24)
        block_back(20, st1)
        for j in range(4):
            tail_r1(st0, j, 0, 512)
        block_front(28)
        block_back(24, st1)
        for j in range(4):
            tail_r2(st0, j, 0, 512)
        block_back(28, st1)
        tail_sig(st0, 0, 512)
        ha, hb = dict(st1), dict(st1)
        tail_a(ha, 0, 256)
        tail_a(hb, 256, 512)
        tail_b(ha, 0, 256)
        tail_b(hb, 256, 512)
        for j in range(4):
            tail_r1(st1, j, 0, 512)
        for j in range(4):
            tail_r2(st1, j, 0, 512)
        tail_sig(st1, 0, 512, table=True)

    nc.compile()
    return nc


def _get_program():
    global _PROGRAM
    if _PROGRAM is None:
        _PROGRAM = _build_program()
    return _PROGRAM


def _host_prep(inputs):
    """Build per-core input maps from the full problem inputs."""
    x = np.asarray(inputs["x"], np.float32)
    y = np.asarray(inputs["y"], np.float32)
    W1 = np.asarray(inputs["W1"], np.float32)
    b1 = np.asarray(inputs["b1"], np.float32)
    Wp = np.asarray(inputs["Wp"], np.float32)
    bp = np.asarray(inputs["bp"], np.float32)
    cbp = np.asarray(inputs["cbp"], np.float32)
    Ws = np.asarray(inputs["Ws"], np.float32)
    bs = np.asarray(inputs["bs"], np.float32)
    cbs = np.asarray(inputs["cbs"], np.float32)
    Wr1 = np.asarray(inputs["Wr1"], np.float32)
    br1 = np.asarray(inputs["br1"], np.float32)
    Wr2 = np.asarray(inputs["Wr2"], np.float32)
    br2 = np.asarray(inputs["br2"], np.float32)
    Wr3 = np.asarray(inputs["Wr3"], np.float32)
    br3 = np.asarray(inputs["br3"], np.float32)

    W1r = W1.reshape(256, 25).T                      # [25 tap, 256 oc]
    W1T = np.concatenate([W1r, np.ones((1, 256), np.float32),
                          b1[None, :]], axis=0) * SA  # [27, 256], x SA
    W1T4 = np.zeros((128, 256), np.float32)
    for qt in range(4):
        W1T4[32 * qt:32 * qt + 27] = W1T
    WT8 = (np.ascontiguousarray(
        Wp.reshape(256, 2, 128, 25).transpose(1, 3, 2, 0)) * SW
    ).reshape(2, 25, 128, 256).transpose(2, 0, 1, 3)  # [128p, 2k, 25t, 256oc]
    WT8 = np.ascontiguousarray(WT8).astype(NP_F8)

    oc = np.arange(128)
    WsT = np.ascontiguousarray(Ws.reshape(16, 8).T[oc % 8])       # [128, 16]
    IND2 = (np.arange(128)[None, :] // 8 == np.arange(16)[:, None]).astype(np.float32)
    INDSQ = np.ascontiguousarray(IND2.T)
    cb1 = np.empty((128, 2), np.float32)
    for m in range(2):
        g = m * 128 + np.arange(128)
        cb1[:, m] = bp[g] / 32.0 + cbp[g // 8, g % 8, 0, 0]
    cb2 = (32.0 * bs + cbs[0, :, 0, 0]).astype(np.float32)

    packr = np.zeros((128, 1024), np.float32)
    packr[0:16, 0:128] = IND2
    packr[32:48, 0:128] = IND2
    for j in range(4):
        packr[j, 128 + 32 * j:128 + 32 * j + 16] = 1.0   # BCIND
        packr[32 * j:32 * j + 16, 256:320] = Wr1.reshape(64, 16).T
        packr[:, 512 + 128 * j + 32 * j] = Wr3.reshape(128)  # WR3T_j
    packr[0:64, 384:512] = Wr2.reshape(128, 64).T        # WR2T (rows 64+ zero)

    packb = np.zeros((128, 644), np.float32)
    packb[:, 0:16] = INDSQ                               # INDSQ m=0 -> parts 0-15
    packb[:, 96:112] = INDSQ                             # INDSQ m=1 -> parts 32-47
    for j in range(4):
        packb[:, 128 + 128 * j + 32 * j:128 + 128 * j + 32 * j + 16] = WsT
        packb[32 * j:32 * j + 16, 640 + j] = 1.0         # INDSEG

    packf = np.zeros((128, 9), np.float32)
    packf[:, 0:2] = cb1
    for j in range(4):
        packf[32 * j:32 * j + 16, 2] = cb2               # CB2V
    packf[0:48, 3] = 1e-9                                # EPS48
    packf[0:4, 4] = 1e-9                                 # EPS4
    packf[0:64, 5] = br1
    packf[:, 6] = br2
    packf[0:97:32, 7] = br3[0]                           # BR3V

    shared = {
        "W1T4": W1T4.astype(NP_BF16),
        "WT8": WT8,
        "PACKR": packr,
        "PACKB": packb.astype(NP_BF16),
        "PACKF": packf,
    }

    in_maps = []
    for c in range(NCORES):
        b, j = divmod(c, NBLK)
        r0 = RB * j
        xpad = np.zeros((H + 8, W + 8), np.float32)
        xpad[4:4 + H, 4:4 + W] = x[b, 0]
        A = np.empty((27, RR, CW), np.float32)
        for dy in range(5):
            for dx in range(5):
                A[dy * 5 + dx] = xpad[r0 + dy:r0 + dy + RR, dx:dx + CW]
        # valid-mask row: -1e30 where the conv1 output position is padding
        rr = np.arange(RR)[:, None]
        cc = np.arange(CW)[None, :]
        valid = (r0 - 2 + rr >= 0) & (r0 - 2 + rr < H) & (cc >= 2) & (cc < 2 + W)
        A[25] = np.where(valid, 0.0, -1e30).astype(np.float32)
        A[26] = 1.0
        m = dict(shared)
        Af = A.reshape(27, AFLAT)
        A4 = np.zeros((128, QW), np.float32)
        for qt in range(4):
            A4[32 * qt:32 * qt + 27] = Af[:, QW * qt:QW * (qt + 1)]
        m["A4"] = A4.astype(NP_BF16)
        m["YV"] = np.ascontiguousarray(y[b, 0, r0:r0 + RB, :].reshape(NPX))
        in_maps.append(m)
    return in_maps


def _gather(results):
    out_seg = np.empty((B, 1, H, W), np.float32)
    out_rec = np.empty((B, 1, H, W), np.float32)
    for c in range(NCORES):
        b, j = divmod(c, NBLK)
        r0 = RB * j
        out_seg[b, 0, r0:r0 + RB, :] = \
            results[c]["OSEG"].astype(np.float32).reshape(RB, W)
        out_rec[b, 0, r0:r0 + RB, :] = \
            results[c]["OREC"].astype(np.float32).reshape(RB, W)
    return out_seg, out_rec


def kernel(**inputs):
    nc = _get_program()
    in_maps = _host_prep(inputs)
    res = run_bass_kernel_spmd(nc, in_maps, list(range(NCORES)))
    return _gather(res.results)
